# revision 3
# baseline (speedup 1.0000x reference)
"""Trainium2 Bass kernel for a naive LSTM (B=64, S=2048, E=H=256).

Strategy (8 NeuronCores, data-parallel over batch, 8 sequences/core):
- Transposed layout: hidden state h^T kept as (128 h-dim partitions, k-chunk x
  batch free). Hidden weights are bf16 stationaries (16 x 128x128), streamed
  through the PE every step; h^T is the tiny moving operand.
- Input projections X @ W_i are computed on-chip in blocks of 256 steps
  (weights stationary, X^T moving at N=256) and stored in SBUF as bf16,
  pre-scaled so that sigmoid(x) = 0.5*tanh(x/2)+0.5 needs one tanh for all
  four gates (i,f,o weights/biases pre-scaled by 0.5 on the host).
- Per step: one identity matmul seeds the PSUM bank with xw_t (off the
  critical path), 16 weight matmuls accumulate W_h h_{t-1}, one ACT tanh over
  all gates, DVE computes sigma/c-update, ACT tanh(c), DVE h = sigma_o*tanh(c)
  written straight into the bf16 output block buffer which doubles as the next
  step's moving operand.
- Output written in transposed layout, reassembled on the host.
"""

import numpy as np
import ml_dtypes

B_TOTAL = 64
S = 2048
E = 256
H = 256
NCORES = 8
B = B_TOTAL // NCORES          # 8 sequences per core
T_BLK = 256
NBLK = S // T_BLK
bf16 = ml_dtypes.bfloat16

_RUNNER_CACHE = {}


# --------------------------------------------------------------------------
# Compatibility patches for the walrus build in this container: it accepts at
# most ONE sync-wait command per instruction; split extras onto NoOps.
# --------------------------------------------------------------------------
def _install_tile_patches():
    import concourse.tile as tile
    import concourse.mybir as mybir
    from concourse.vector_clock import ScopedClock

    if getattr(tile.TileContext, "_lstm_patched", False):
        return

    def _patched_drain_and_barrier(self, tick_clock, wait_clock):
        nc = self.nc
        probe = nc.sync.nop()
        wait_clock.add_sem_waits(probe.ins, ScopedClock({None: tick_clock.global_clock}))
        si = probe.ins.sync_info
        waits = list(si.on_wait) if si and si.on_wait else []
        if len(waits) > 1:
            si.on_wait = waits[:1]
            for w in waits[1:]:
                extra = nc.sync.nop()
                esi = extra.ins.sync_info
                if esi is None:
                    extra.ins.sync_info = mybir.SyncInfo(on_wait=[w], on_update=[])
                else:
                    esi.on_wait = [w]
        nc.sync.drain()
        nc.all_engine_barrier()
        popped = nc._tile_sem_poison_stack.pop()
        assert popped is self._sem_poison
        nc.clear_and_free_semaphores(list(self.sems.allocated().values()))
        nc.all_engine_barrier()

    tile.TileContext._drain_and_barrier = _patched_drain_and_barrier
    tile.TileContext._lstm_patched = True


_split_counter = [0]


def _split_sync_waits(nc, max_waits=1):
    import concourse.mybir as mybir

    for f in nc.m.functions:
        for blk in f.blocks:
            insts = blk.instructions
            out = []
            dirty = False
            for inst in insts:
                si = inst.sync_info
                waits = list(si.on_wait) if si and si.on_wait else []
                if len(waits) > max_waits:
                    for w in waits[:-max_waits]:
                        _split_counter[0] += 1
                        nop = mybir.InstNoOp(
                            name=f"_swsplit{_split_counter[0]}", ins=[], outs=[])
                        nop.engine = inst.engine
                        nop.sync_info = mybir.SyncInfo(on_wait=[w], on_update=[])
                        out.append(nop)
                    si.on_wait = waits[-max_waits:]
                    dirty = True
                out.append(inst)
            if dirty:
                blk.instructions = out


# --------------------------------------------------------------------------
# Device program
# --------------------------------------------------------------------------
def _build_module():
    import concourse.bass as bass
    import concourse.tile as tile
    import concourse.mybir as mybir

    _install_tile_patches()
    dt = mybir.dt
    AF = mybir.ActivationFunctionType
    ALU = mybir.AluOpType

    nc = bass.Bass()
    # Inputs (per core)
    wh_d = nc.dram_tensor("wh", [128, 16 * 128], dt.bfloat16, kind="ExternalInput")
    wi_d = nc.dram_tensor("wi", [128, 16 * 128], dt.bfloat16, kind="ExternalInput")
    bias_d = nc.dram_tensor("bias", [128, 8], dt.float32, kind="ExternalInput")
    ident_d = nc.dram_tensor("ident", [128, 128], dt.bfloat16, kind="ExternalInput")
    xt_d = nc.dram_tensor("xt", [B, 2, 128, S], dt.bfloat16, kind="ExternalInput")
    # Outputs
    y_d = nc.dram_tensor("y", [NBLK, 128, T_BLK * 2 * B], dt.bfloat16,
                         kind="ExternalOutput")
    c_d = nc.dram_tensor("c", [128, 2 * B], dt.float32, kind="ExternalOutput")

    # weight tile index within packed buffers: (m, k, g) -> col offset
    def widx(m, k, g):
        return ((m * 2 + k) * 4 + g) * 128

    with tile.TileContext(nc) as tc:
        with (
            tc.tile_pool(name="consts", bufs=1) as consts,
            tc.tile_pool(name="xwp", bufs=2) as xwpool,
            tc.tile_pool(name="xtp", bufs=4) as xtpool,
            tc.tile_pool(name="yp", bufs=2) as ypool,
            tc.tile_pool(name="state", bufs=1) as spool,
            tc.tile_pool(name="work", bufs=6) as work,
            tc.tile_pool(name="rpsum", bufs=5, space="PSUM") as rpsum,
            tc.tile_pool(name="ppsum", bufs=2, space="PSUM") as ppsum,
        ):
            whs = consts.tile([128, 16 * 128], dt.bfloat16)
            nc.sync.dma_start(whs[:], wh_d[:])
            wis = consts.tile([128, 16 * 128], dt.bfloat16)
            nc.sync.dma_start(wis[:], wi_d[:])
            biast = consts.tile([128, 8], dt.float32)
            nc.sync.dma_start(biast[:], bias_d[:])
            idt = consts.tile([128, 128], dt.bfloat16)
            nc.sync.dma_start(idt[:], ident_d[:])

            cst = spool.tile([128, 2 * B], dt.float32)
            h0 = spool.tile([128, 2 * B], dt.bfloat16)
            nc.vector.memset(cst[:], 0.0)
            nc.vector.memset(h0[:], 0.0)

            # ------------------------------------------------------------
            # Phase-1 unit: compute XWT for (block, batch b, half m) into the
            # given SBUF buffer: for each gate g accumulate over k the matmul
            # Wi[(m,k,g)].T @ XT[b,k,:,t0:t0+T_BLK], add bias, scale already
            # folded; write bf16 strided into xw block layout
            # [t][g*2+m][b] (free idx = t*64 + (g*2+m)*8 + b).
            # ------------------------------------------------------------
            def phase1_unit(xwbuf, blk, b, m):
                t0 = blk * T_BLK
                xtk = []
                for k in range(2):
                    xk = xtpool.tile([128, T_BLK], dt.bfloat16, tag=f"xt{k}")
                    nc.sync.dma_start(xk[:], xt_d[b, k, :, t0:t0 + T_BLK])
                    xtk.append(xk)
                for g in range(4):
                    P = ppsum.tile([128, T_BLK], dt.float32, tag="p1")
                    for k in range(2):
                        nc.tensor.matmul(
                            P[:], wis[:, widx(m, k, g):widx(m, k, g) + 128],
                            xtk[k][:], start=(k == 0), stop=(k == 1),
                            skip_group_check=True)
                    dst = xwbuf[:, (g * 2 + m) * 8 + b::64]
                    nc.scalar.activation(dst, P[:], AF.Identity,
                                         bias=biast[:, m * 4 + g:m * 4 + g + 1],
                                         scale=1.0)

            # ------------------------------------------------------------
            # Recurrence step
            # ------------------------------------------------------------
            def rec_step(xwbuf, ybuf, t, hprev):
                P = rpsum.tile([128, 8 * B], dt.float32, tag="rp")
                nc.tensor.matmul(P[:], idt[:], xwbuf[:, t * 8 * B:(t + 1) * 8 * B],
                                 start=True, stop=False, skip_group_check=True)
                for k in range(2):
                    for g in range(4):
                        for m in range(2):
                            sl = (g * 2 + m) * B
                            nc.tensor.matmul(
                                P[:, sl:sl + B],
                                whs[:, widx(m, k, g):widx(m, k, g) + 128],
                                hprev[:, k * B:(k + 1) * B],
                                start=False, stop=(k == 1),
                                skip_group_check=True)
                Tall = work.tile([128, 8 * B], dt.float32, tag="tall")
                nc.scalar.activation(Tall[:], P[:], AF.Tanh)
                Sg = work.tile([128, 6 * B], dt.float32, tag="sig")
                nc.vector.tensor_scalar(Sg[:], Tall[:, 0:6 * B], 0.5, 0.5,
                                        ALU.mult, ALU.add)
                t1 = work.tile([128, 2 * B], dt.float32, tag="t1")
                nc.vector.tensor_mul(t1[:], Sg[:, 2 * B:4 * B], cst[:])
                t2 = work.tile([128, 2 * B], dt.float32, tag="t2")
                nc.vector.tensor_mul(t2[:], Sg[:, 0:2 * B], Tall[:, 6 * B:8 * B])
                nc.vector.tensor_add(cst[:], t1[:], t2[:])
                tc_t = work.tile([128, 2 * B], dt.float32, tag="tc")
                nc.scalar.activation(tc_t[:], cst[:], AF.Tanh)
                hnew = ybuf[:, t * 2 * B:(t + 1) * 2 * B]
                # split by k-chunk so next step's k=0 matmuls can start early
                nc.vector.tensor_mul(hnew[:, 0:B], Sg[:, 4 * B:5 * B], tc_t[:, 0:B])
                nc.vector.tensor_mul(hnew[:, B:2 * B], Sg[:, 5 * B:6 * B],
                                     tc_t[:, B:2 * B])
                return hnew

            # ------------------------------------------------------------
            # Main schedule: for each block, interleave next block's phase-1
            # units between recurrence steps.
            # ------------------------------------------------------------
            xwbufs = [None, None]
            # prologue: phase-1 for block 0
            xwbufs[0] = xwpool.tile([128, T_BLK * 8 * B], dt.bfloat16, tag="xw", name="xwbuf0")
            for b in range(B):
                for m in range(2):
                    phase1_unit(xwbufs[0], 0, b, m)

            hprev = h0[:]
            for blk in range(NBLK):
                ybuf = ypool.tile([128, T_BLK * 2 * B], dt.bfloat16, tag="y")
                if blk + 1 < NBLK:
                    xwbufs[(blk + 1) % 2] = xwpool.tile(
                        [128, T_BLK * 8 * B], dt.bfloat16, tag="xw",
                        name=f"xwbuf{blk + 1}")
                p1_units = ([(b, m) for b in range(B) for m in range(2)]
                            if blk + 1 < NBLK else [])
                xwb = xwbufs[blk % 2]
                for t in range(T_BLK):
                    hprev = rec_step(xwb, ybuf, t, hprev)
                    if t % 16 == 0 and p1_units:
                        b, m = p1_units.pop(0)
                        phase1_unit(xwbufs[(blk + 1) % 2], blk + 1, b, m)
                nc.sync.dma_start(y_d[blk, :, :], ybuf[:])
            nc.sync.dma_start(c_d[:], cst[:])

    _split_sync_waits(nc)
    return nc


def _get_runner():
    if "runner" in _RUNNER_CACHE:
        return _RUNNER_CACHE["runner"]
    import jax
    from jax.sharding import Mesh, PartitionSpec
    from jax.experimental.shard_map import shard_map
    import concourse.mybir as mybir
    from concourse import bass2jax
    from concourse.bass2jax import _bass_exec_p, install_neuronx_cc_hook

    nc = _build_module()
    install_neuronx_cc_hook()
    partition_name = nc.partition_id_tensor.name if nc.partition_id_tensor else None
    in_names, out_names, out_avals, zero_shapes = [], [], [], []
    for alloc in nc.m.functions[0].allocations:
        if not isinstance(alloc, mybir.MemoryLocationSet):
            continue
        name = alloc.memorylocations[0].name
        if alloc.kind == "ExternalInput":
            if name != partition_name:
                in_names.append(name)
        elif alloc.kind == "ExternalOutput":
            shape = tuple(alloc.tensor_shape)
            dtype = mybir.dt.np(alloc.dtype)
            out_names.append(name)
            out_avals.append(jax.core.ShapedArray(shape, dtype))
            zero_shapes.append((shape, dtype))
    n_params = len(in_names)
    n_outs = len(out_avals)
    in_names_all = list(in_names) + out_names
    if partition_name is not None:
        in_names_all.append(partition_name)
    donate = tuple(range(n_params, n_params + n_outs))

    def _body(*args):
        operands = list(args)
        if partition_name is not None:
            operands.append(bass2jax.partition_id_tensor())
        outs = _bass_exec_p.bind(
            *operands, out_avals=tuple(out_avals), in_names=tuple(in_names_all),
            out_names=tuple(out_names), lowering_input_output_aliases=(),
            sim_require_finite=True, sim_require_nnan=True, nc=nc)
        return tuple(outs)

    devices = jax.devices()[:NCORES]
    mesh = Mesh(np.asarray(devices), ("core",))
    in_specs = (PartitionSpec("core"),) * (n_params + n_outs)
    out_specs = (PartitionSpec("core"),) * n_outs
    sharded = jax.jit(shard_map(_body, mesh=mesh, in_specs=in_specs,
                                out_specs=out_specs, check_rep=False),
                      donate_argnums=donate, keep_unused=True)

    def run(in_maps):
        concat_in = [np.concatenate([np.asarray(in_maps[c][nm])
                                     for c in range(NCORES)], axis=0)
                     for nm in in_names]
        zeros = [np.zeros((NCORES * s[0],) + tuple(s[1:]), d)
                 for s, d in zero_shapes]
        out = sharded(*concat_in, *zeros)
        import jax as _jax
        _jax.block_until_ready(out)
        return [
            {nm: np.asarray(out[i]).reshape(NCORES, *out_avals[i].shape)[c]
             for i, nm in enumerate(out_names)}
            for c in range(NCORES)
        ]

    _RUNNER_CACHE["runner"] = run
    return run


# --------------------------------------------------------------------------
# Host-side packing / unpacking
# --------------------------------------------------------------------------
def _pack_weights(W_ii, W_hi, b_i, W_if, W_hf, b_f, W_ig, W_hg, b_g,
                  W_io, W_ho, b_o):
    # gate order [i, f, o, g], sigmoid gates pre-scaled by 0.5
    gates = [(W_ii, W_hi, b_i, 0.5), (W_if, W_hf, b_f, 0.5),
             (W_io, W_ho, b_o, 0.5), (W_ig, W_hg, b_g, 1.0)]
    wh = np.zeros((128, 16 * 128), np.float32)
    wi = np.zeros((128, 16 * 128), np.float32)
    bias = np.zeros((128, 8), np.float32)
    for g, (Wx, Wh, bb, s) in enumerate(gates):
        for m in range(2):
            for k in range(2):
                off = ((m * 2 + k) * 4 + g) * 128
                wh[:, off:off + 128] = s * Wh[k * 128:(k + 1) * 128,
                                              m * 128:(m + 1) * 128]
                wi[:, off:off + 128] = s * Wx[k * 128:(k + 1) * 128,
                                              m * 128:(m + 1) * 128]
            bias[:, m * 4 + g] = s * bb[m * 128:(m + 1) * 128]
    return wh.astype(bf16), wi.astype(bf16), bias


def kernel(X, W_ii, W_hi, b_i, W_if, W_hf, b_f, W_ig, W_hg, b_g,
           W_io, W_ho, b_o):
    X = np.asarray(X, np.float32)
    args = [np.asarray(a, np.float32) for a in
            (W_ii, W_hi, b_i, W_if, W_hf, b_f, W_ig, W_hg, b_g, W_io, W_ho, b_o)]
    wh, wi, bias = _pack_weights(*args)
    ident = np.eye(128, dtype=np.float32).astype(bf16)

    # per-core inputs: X^T in bf16, (B, 2, 128, S)
    in_maps = []
    for c in range(NCORES):
        xc = X[c * B:(c + 1) * B]                       # (B, S, E)
        xt = np.ascontiguousarray(xc.transpose(0, 2, 1))  # (B, E, S)
        xt = xt.reshape(B, 2, 128, S).astype(bf16)
        in_maps.append({"wh": wh, "wi": wi, "bias": bias, "ident": ident,
                        "xt": xt})

    run = _get_runner()
    results = run(in_maps)

    hidden = np.empty((B_TOTAL, S, H), np.float32)
    c_fin = np.empty((B_TOTAL, H), np.float32)
    for c in range(NCORES):
        y = results[c]["y"].astype(np.float32)          # (NBLK, 128, T_BLK*16)
        y = y.reshape(NBLK, 128, T_BLK, 2, B)           # [blk, p, t, k, b]
        # hidden[b, blk*T+t, k*128+p]
        y = y.transpose(4, 0, 2, 3, 1)                  # (b, blk, t, k, p)
        hidden[c * B:(c + 1) * B] = y.reshape(B, S, H)
        cc = results[c]["c"]                            # (128, 2*B): [p, k*B+b]
        cc = cc.reshape(128, 2, B).transpose(2, 1, 0)   # (b, k, p)
        c_fin[c * B:(c + 1) * B] = cc.reshape(B, H)
    h_fin = hidden[:, -1, :].copy()
    return hidden, (h_fin, c_fin)


# revision 4
# speedup vs baseline: 1.0260x; 1.0260x over previous
"""Trainium2 Bass kernel for a naive LSTM (B=64, S=2048, E=H=256).

Strategy (8 NeuronCores, data-parallel over batch, 8 sequences/core):
- Transposed layout: hidden state h^T kept as (128 h-dim partitions, k-chunk x
  batch free). Hidden weights are bf16 stationaries (16 x 128x128), streamed
  through the PE every step; h^T is the tiny moving operand.
- Input projections X @ W_i are computed on-chip in blocks of 256 steps
  (weights stationary, X^T moving at N=256) and stored in SBUF as bf16,
  pre-scaled so that sigmoid(x) = 0.5*tanh(x/2)+0.5 needs one tanh for all
  four gates (i,f,o weights/biases pre-scaled by 0.5 on the host).
- Per step: one identity matmul seeds the PSUM bank with xw_t (off the
  critical path), 16 weight matmuls accumulate W_h h_{t-1}, one ACT tanh over
  all gates, DVE computes sigma/c-update, ACT tanh(c), DVE h = sigma_o*tanh(c)
  written straight into the bf16 output block buffer which doubles as the next
  step's moving operand.
- Output written in transposed layout, reassembled on the host.
"""

import numpy as np
import ml_dtypes

B_TOTAL = 64
S = 2048
E = 256
H = 256
NCORES = 8
B = B_TOTAL // NCORES          # 8 sequences per core
T_BLK = 256
NBLK = S // T_BLK
bf16 = ml_dtypes.bfloat16

_RUNNER_CACHE = {}


# --------------------------------------------------------------------------
# Compatibility patches for the walrus build in this container: it accepts at
# most ONE sync-wait command per instruction; split extras onto NoOps.
# --------------------------------------------------------------------------
def _install_tile_patches():
    import concourse.tile as tile
    import concourse.mybir as mybir
    from concourse.vector_clock import ScopedClock

    if getattr(tile.TileContext, "_lstm_patched", False):
        return

    def _patched_drain_and_barrier(self, tick_clock, wait_clock):
        nc = self.nc
        probe = nc.sync.nop()
        wait_clock.add_sem_waits(probe.ins, ScopedClock({None: tick_clock.global_clock}))
        si = probe.ins.sync_info
        waits = list(si.on_wait) if si and si.on_wait else []
        if len(waits) > 1:
            si.on_wait = waits[:1]
            for w in waits[1:]:
                extra = nc.sync.nop()
                esi = extra.ins.sync_info
                if esi is None:
                    extra.ins.sync_info = mybir.SyncInfo(on_wait=[w], on_update=[])
                else:
                    esi.on_wait = [w]
        nc.sync.drain()
        nc.all_engine_barrier()
        popped = nc._tile_sem_poison_stack.pop()
        assert popped is self._sem_poison
        nc.clear_and_free_semaphores(list(self.sems.allocated().values()))
        nc.all_engine_barrier()

    tile.TileContext._drain_and_barrier = _patched_drain_and_barrier
    tile.TileContext._lstm_patched = True


_split_counter = [0]


def _split_sync_waits(nc, max_waits=1):
    import concourse.mybir as mybir

    for f in nc.m.functions:
        for blk in f.blocks:
            insts = blk.instructions
            out = []
            dirty = False
            for inst in insts:
                si = inst.sync_info
                waits = list(si.on_wait) if si and si.on_wait else []
                if len(waits) > max_waits:
                    for w in waits[:-max_waits]:
                        _split_counter[0] += 1
                        nop = mybir.InstNoOp(
                            name=f"_swsplit{_split_counter[0]}", ins=[], outs=[])
                        nop.engine = inst.engine
                        nop.sync_info = mybir.SyncInfo(on_wait=[w], on_update=[])
                        out.append(nop)
                    si.on_wait = waits[-max_waits:]
                    dirty = True
                out.append(inst)
            if dirty:
                blk.instructions = out


# --------------------------------------------------------------------------
# Device program
# --------------------------------------------------------------------------
def _build_module():
    import concourse.bass as bass
    import concourse.tile as tile
    import concourse.mybir as mybir

    _install_tile_patches()
    dt = mybir.dt
    AF = mybir.ActivationFunctionType
    ALU = mybir.AluOpType

    nc = bass.Bass()
    # Inputs (per core)
    wh_d = nc.dram_tensor("wh", [128, 16 * 128], dt.bfloat16, kind="ExternalInput")
    wi_d = nc.dram_tensor("wi", [128, 16 * 128], dt.bfloat16, kind="ExternalInput")
    bias_d = nc.dram_tensor("bias", [128, 8], dt.float32, kind="ExternalInput")
    ident_d = nc.dram_tensor("ident", [128, 128], dt.bfloat16, kind="ExternalInput")
    xt_d = nc.dram_tensor("xt", [B, 2, 128, S], dt.bfloat16, kind="ExternalInput")
    # Outputs
    y_d = nc.dram_tensor("y", [NBLK, 128, T_BLK * 2 * B], dt.bfloat16,
                         kind="ExternalOutput")
    c_d = nc.dram_tensor("c", [128, 2 * B], dt.float32, kind="ExternalOutput")

    # weight tile index within packed buffers: (m, k, g) -> col offset
    def widx(m, k, g):
        return ((m * 2 + k) * 4 + g) * 128

    with tile.TileContext(nc) as tc:
        with (
            tc.tile_pool(name="consts", bufs=1) as consts,
            tc.tile_pool(name="xwp", bufs=2) as xwpool,
            tc.tile_pool(name="xtp", bufs=4) as xtpool,
            tc.tile_pool(name="yp", bufs=2) as ypool,
            tc.tile_pool(name="state", bufs=1) as spool,
            tc.tile_pool(name="work", bufs=6) as work,
            tc.tile_pool(name="rpsum", bufs=5, space="PSUM") as rpsum,
            tc.tile_pool(name="ppsum", bufs=2, space="PSUM") as ppsum,
        ):
            whs = consts.tile([128, 16 * 128], dt.bfloat16)
            nc.sync.dma_start(whs[:], wh_d[:])
            wis = consts.tile([128, 16 * 128], dt.bfloat16)
            nc.sync.dma_start(wis[:], wi_d[:])
            biast = consts.tile([128, 8], dt.float32)
            nc.sync.dma_start(biast[:], bias_d[:])
            idt = consts.tile([128, 128], dt.bfloat16)
            nc.sync.dma_start(idt[:], ident_d[:])

            cst = spool.tile([128, 2 * B], dt.float32)
            h0 = spool.tile([128, 2 * B], dt.bfloat16)
            nc.vector.memset(cst[:], 0.0)
            nc.vector.memset(h0[:], 0.0)

            # ------------------------------------------------------------
            # Phase-1 unit: compute XWT for (block, batch b, half m) into the
            # given SBUF buffer: for each gate g accumulate over k the matmul
            # Wi[(m,k,g)].T @ XT[b,k,:,t0:t0+T_BLK], add bias, scale already
            # folded; write bf16 strided into xw block layout
            # [t][g*2+m][b] (free idx = t*64 + (g*2+m)*8 + b).
            # ------------------------------------------------------------
            def phase1_unit(xwbuf, blk, b, m):
                t0 = blk * T_BLK
                xtk = []
                for k in range(2):
                    xk = xtpool.tile([128, T_BLK], dt.bfloat16, tag=f"xt{k}")
                    nc.sync.dma_start(xk[:], xt_d[b, k, :, t0:t0 + T_BLK])
                    xtk.append(xk)
                for g in range(4):
                    P = ppsum.tile([128, T_BLK], dt.float32, tag="p1")
                    for k in range(2):
                        nc.tensor.matmul(
                            P[:], wis[:, widx(m, k, g):widx(m, k, g) + 128],
                            xtk[k][:], start=(k == 0), stop=(k == 1),
                            skip_group_check=True)
                    dst = xwbuf[:, (g * 2 + m) * 8 + b::64]
                    nc.scalar.activation(dst, P[:], AF.Identity,
                                         bias=biast[:, m * 4 + g:m * 4 + g + 1],
                                         scale=1.0)

            # ------------------------------------------------------------
            # Recurrence step
            # ------------------------------------------------------------
            def rec_step(xwbuf, ybuf, t, hprev):
                P = rpsum.tile([128, 8 * B], dt.float32, tag="rp")
                nc.tensor.matmul(P[:], idt[:], xwbuf[:, t * 8 * B:(t + 1) * 8 * B],
                                 start=True, stop=False, skip_group_check=True)
                for k in range(2):
                    for g in range(4):
                        for m in range(2):
                            sl = (g * 2 + m) * B
                            nc.tensor.matmul(
                                P[:, sl:sl + B],
                                whs[:, widx(m, k, g):widx(m, k, g) + 128],
                                hprev[:, k * B:(k + 1) * B],
                                start=False, stop=(k == 1),
                                skip_group_check=True)
                Tall = work.tile([128, 8 * B], dt.float32, tag="tall")
                nc.scalar.activation(Tall[:], P[:], AF.Tanh)
                # c-path via fused STT ops (no sigma stage on the cycle):
                # u = (Tf+1)*c = 2*sigma_f*c ; v = (Ti+1)*Tg = 2*sigma_i*g
                # s = u+v = 2*c_new ; tanh(c_new) = Tanh(0.5*s); c = 0.5*s
                u = work.tile([128, 2 * B], dt.float32, tag="t1")
                nc.vector.scalar_tensor_tensor(
                    u[:], Tall[:, 2 * B:4 * B], 1.0, cst[:], ALU.add, ALU.mult)
                v = work.tile([128, 2 * B], dt.float32, tag="t2")
                nc.vector.scalar_tensor_tensor(
                    v[:], Tall[:, 0:2 * B], 1.0, Tall[:, 6 * B:8 * B],
                    ALU.add, ALU.mult)
                s2 = work.tile([128, 2 * B], dt.float32, tag="s2")
                nc.vector.tensor_add(s2[:], u[:], v[:])
                tc_t = work.tile([128, 2 * B], dt.float32, tag="tc")
                nc.scalar.activation(tc_t[:], s2[:], AF.Tanh, scale=0.5)
                nc.vector.tensor_scalar(cst[:], s2[:], 0.5, None, ALU.mult)
                Sg = work.tile([128, 2 * B], dt.float32, tag="sig")
                nc.vector.tensor_scalar(Sg[:], Tall[:, 4 * B:6 * B], 0.5, 0.5,
                                        ALU.mult, ALU.add)
                hnew = ybuf[:, t * 2 * B:(t + 1) * 2 * B]
                # split by k-chunk so next step's k=0 matmuls can start early
                nc.vector.tensor_mul(hnew[:, 0:B], Sg[:, 0:B], tc_t[:, 0:B])
                nc.vector.tensor_mul(hnew[:, B:2 * B], Sg[:, B:2 * B],
                                     tc_t[:, B:2 * B])
                return hnew

            # ------------------------------------------------------------
            # Main schedule: for each block, interleave next block's phase-1
            # units between recurrence steps.
            # ------------------------------------------------------------
            xwbufs = [None, None]
            # prologue: phase-1 for block 0
            xwbufs[0] = xwpool.tile([128, T_BLK * 8 * B], dt.bfloat16, tag="xw", name="xwbuf0")
            for b in range(B):
                for m in range(2):
                    phase1_unit(xwbufs[0], 0, b, m)

            hprev = h0[:]
            for blk in range(NBLK):
                ybuf = ypool.tile([128, T_BLK * 2 * B], dt.bfloat16, tag="y")
                if blk + 1 < NBLK:
                    xwbufs[(blk + 1) % 2] = xwpool.tile(
                        [128, T_BLK * 8 * B], dt.bfloat16, tag="xw",
                        name=f"xwbuf{blk + 1}")
                p1_units = ([(b, m) for b in range(B) for m in range(2)]
                            if blk + 1 < NBLK else [])
                xwb = xwbufs[blk % 2]
                for t in range(T_BLK):
                    hprev = rec_step(xwb, ybuf, t, hprev)
                    if t % 16 == 0 and p1_units:
                        b, m = p1_units.pop(0)
                        phase1_unit(xwbufs[(blk + 1) % 2], blk + 1, b, m)
                nc.sync.dma_start(y_d[blk, :, :], ybuf[:])
            nc.sync.dma_start(c_d[:], cst[:])

    _split_sync_waits(nc)
    return nc


def _get_runner():
    if "runner" in _RUNNER_CACHE:
        return _RUNNER_CACHE["runner"]
    import jax
    from jax.sharding import Mesh, PartitionSpec
    from jax.experimental.shard_map import shard_map
    import concourse.mybir as mybir
    from concourse import bass2jax
    from concourse.bass2jax import _bass_exec_p, install_neuronx_cc_hook

    nc = _build_module()
    install_neuronx_cc_hook()
    partition_name = nc.partition_id_tensor.name if nc.partition_id_tensor else None
    in_names, out_names, out_avals, zero_shapes = [], [], [], []
    for alloc in nc.m.functions[0].allocations:
        if not isinstance(alloc, mybir.MemoryLocationSet):
            continue
        name = alloc.memorylocations[0].name
        if alloc.kind == "ExternalInput":
            if name != partition_name:
                in_names.append(name)
        elif alloc.kind == "ExternalOutput":
            shape = tuple(alloc.tensor_shape)
            dtype = mybir.dt.np(alloc.dtype)
            out_names.append(name)
            out_avals.append(jax.core.ShapedArray(shape, dtype))
            zero_shapes.append((shape, dtype))
    n_params = len(in_names)
    n_outs = len(out_avals)
    in_names_all = list(in_names) + out_names
    if partition_name is not None:
        in_names_all.append(partition_name)
    donate = tuple(range(n_params, n_params + n_outs))

    def _body(*args):
        operands = list(args)
        if partition_name is not None:
            operands.append(bass2jax.partition_id_tensor())
        outs = _bass_exec_p.bind(
            *operands, out_avals=tuple(out_avals), in_names=tuple(in_names_all),
            out_names=tuple(out_names), lowering_input_output_aliases=(),
            sim_require_finite=True, sim_require_nnan=True, nc=nc)
        return tuple(outs)

    devices = jax.devices()[:NCORES]
    mesh = Mesh(np.asarray(devices), ("core",))
    in_specs = (PartitionSpec("core"),) * (n_params + n_outs)
    out_specs = (PartitionSpec("core"),) * n_outs
    sharded = jax.jit(shard_map(_body, mesh=mesh, in_specs=in_specs,
                                out_specs=out_specs, check_rep=False),
                      donate_argnums=donate, keep_unused=True)

    def run(in_maps):
        concat_in = [np.concatenate([np.asarray(in_maps[c][nm])
                                     for c in range(NCORES)], axis=0)
                     for nm in in_names]
        zeros = [np.zeros((NCORES * s[0],) + tuple(s[1:]), d)
                 for s, d in zero_shapes]
        out = sharded(*concat_in, *zeros)
        import jax as _jax
        _jax.block_until_ready(out)
        return [
            {nm: np.asarray(out[i]).reshape(NCORES, *out_avals[i].shape)[c]
             for i, nm in enumerate(out_names)}
            for c in range(NCORES)
        ]

    _RUNNER_CACHE["runner"] = run
    return run


# --------------------------------------------------------------------------
# Host-side packing / unpacking
# --------------------------------------------------------------------------
def _pack_weights(W_ii, W_hi, b_i, W_if, W_hf, b_f, W_ig, W_hg, b_g,
                  W_io, W_ho, b_o):
    # gate order [i, f, o, g], sigmoid gates pre-scaled by 0.5
    gates = [(W_ii, W_hi, b_i, 0.5), (W_if, W_hf, b_f, 0.5),
             (W_io, W_ho, b_o, 0.5), (W_ig, W_hg, b_g, 1.0)]
    wh = np.zeros((128, 16 * 128), np.float32)
    wi = np.zeros((128, 16 * 128), np.float32)
    bias = np.zeros((128, 8), np.float32)
    for g, (Wx, Wh, bb, s) in enumerate(gates):
        for m in range(2):
            for k in range(2):
                off = ((m * 2 + k) * 4 + g) * 128
                wh[:, off:off + 128] = s * Wh[k * 128:(k + 1) * 128,
                                              m * 128:(m + 1) * 128]
                wi[:, off:off + 128] = s * Wx[k * 128:(k + 1) * 128,
                                              m * 128:(m + 1) * 128]
            bias[:, m * 4 + g] = s * bb[m * 128:(m + 1) * 128]
    return wh.astype(bf16), wi.astype(bf16), bias


def kernel(X, W_ii, W_hi, b_i, W_if, W_hf, b_f, W_ig, W_hg, b_g,
           W_io, W_ho, b_o):
    X = np.asarray(X, np.float32)
    args = [np.asarray(a, np.float32) for a in
            (W_ii, W_hi, b_i, W_if, W_hf, b_f, W_ig, W_hg, b_g, W_io, W_ho, b_o)]
    wh, wi, bias = _pack_weights(*args)
    ident = np.eye(128, dtype=np.float32).astype(bf16)

    # per-core inputs: X^T in bf16, (B, 2, 128, S)
    in_maps = []
    for c in range(NCORES):
        xc = X[c * B:(c + 1) * B]                       # (B, S, E)
        xt = np.ascontiguousarray(xc.transpose(0, 2, 1))  # (B, E, S)
        xt = xt.reshape(B, 2, 128, S).astype(bf16)
        in_maps.append({"wh": wh, "wi": wi, "bias": bias, "ident": ident,
                        "xt": xt})

    run = _get_runner()
    results = run(in_maps)

    hidden = np.empty((B_TOTAL, S, H), np.float32)
    c_fin = np.empty((B_TOTAL, H), np.float32)
    for c in range(NCORES):
        y = results[c]["y"].astype(np.float32)          # (NBLK, 128, T_BLK*16)
        y = y.reshape(NBLK, 128, T_BLK, 2, B)           # [blk, p, t, k, b]
        # hidden[b, blk*T+t, k*128+p]
        y = y.transpose(4, 0, 2, 3, 1)                  # (b, blk, t, k, p)
        hidden[c * B:(c + 1) * B] = y.reshape(B, S, H)
        cc = results[c]["c"]                            # (128, 2*B): [p, k*B+b]
        cc = cc.reshape(128, 2, B).transpose(2, 1, 0)   # (b, k, p)
        c_fin[c * B:(c + 1) * B] = cc.reshape(B, H)
    h_fin = hidden[:, -1, :].copy()
    return hidden, (h_fin, c_fin)
